# revision 19
# baseline (speedup 1.0000x reference)
"""Trainium2 Bass kernel for nn_CollectiveDecActorTaxi0Obs (gnn_message_passing).

Computes, for obs [32768, 48], per-zone dense heads W [81, 48, 5] (+bias b,
adjacency idx/mask [81, 5]):
    logits = einsum('bd,ndk->bnk', obs, W) + b ; masked softmax over k
    out[b, n, idx[n, k]] += probs[b, n, k]              -> [32768, 81, 81] f32

Strategy (pure data parallelism, 8 cores, batch-sharded 4096 rows each):
  The [B, 81, 81] output is >93% structural zeros: per batch row only the
  <=405 (n, idx[n,k]) positions can be nonzero, and WHICH positions is
  batch-independent (depends only on idx/mask). The device computes just
  the compact masked-softmax probs for the valid (n,k) slots — [nvalid<=405,
  B] — and the host scatters those columns into the zeroed full [B, 6561]
  output (summing rare duplicate (n, idx) pairs). This cuts device HBM
  writes ~30x vs the dense output and removes the scatter matmuls.

  Device math (fp16 operands, batch on the free dim, 1024-column blocks;
  fp16 matmul runs at bf16 speed and the 16-bit moving operand allows
  N=1024 per matmul, halving instruction count vs fp32):
    - logits:  per 128-slot "pair", one matmul: stationary [49, pw] holds
               fp16 W columns per slot plus a bias row; moving [49, 1024]
               holds fp16 x plus a ones row. fp16 rounding of W/x gives
               ~3e-4 logit error (vs 2e-2 tolerance).
    - exp:     scalar-engine activation, writing fp16. The rounding of exp
               appears in BOTH numerator and denominator of the softmax and
               largely cancels in probs.
    - denom:   0/1 matmul ob [pw, 81] sums each zone's exp over its valid
               slots (masked slots have ob=0 so their exp is irrelevant).
    - recip:   vector-engine reciprocal_approx_fast (f32, ~18 bits), cast
               to fp16.
    - expand:  0/1 matmul E [81, pw] broadcasts zone recips to slot rows;
               probs = exp * recip on the vector engine, written fp16 and
               DMA'd per (pair, block) — SBUF->DRAM stores spray rows
               across all 16 DMA engines, so stores run near full BW.
  Input is loaded as 8 separate DRAM params (one per 512-col slice):
  DRAM->SBUF loads pin to one DMA engine *per param*, so splitting spreads
  the load across 8 engines (~3 us instead of ~35 us serialized).
"""

import os
import sys

sys.path.insert(0, "/opt/trn_rl_repo")

import numpy as np

NZ = 81          # zones
D = 48           # obs dim used
DA = D + 1       # + ones row for bias
KADJ = 5         # adjacency slots per zone
NCORES = 8
BATCH = 32768
BLOC = BATCH // NCORES   # 4096 rows per core
BF = 512                 # batch free-dim block (f32 PSUM bank = 512 cols)
NLOAD = 8                # input split into this many DRAM params
P = 128

LAST_RESULTS = None


def _build_consts(W, b, idx, mask):
    """Pack valid (zone, slot) pairs into sequential rows; build fp16 consts."""
    W = np.asarray(W, np.float32)
    b = np.asarray(b, np.float32)
    idx = np.asarray(idx)
    mask = np.asarray(mask, np.float32)

    slots = [(n, k) for n in range(NZ) for k in range(KADJ) if mask[n, k] > 0]
    nvalid = len(slots)
    npairs = (nvalid + P - 1) // P
    pw = [min(P, nvalid - P * p) for p in range(npairs)]

    Wa = np.zeros((DA, npairs * P), np.float16)
    ob = [np.zeros((pw[p], NZ), np.float16) for p in range(npairs)]
    E = [np.zeros((NZ, pw[p]), np.float16) for p in range(npairs)]
    cols = np.empty(nvalid, np.int64)

    for s, (n, k) in enumerate(slots):
        Wa[:D, s] = W[n, :, k].astype(np.float16)
        Wa[D, s] = np.float16(b[n, k])
        p, r = divmod(s, P)
        ob[p][r, n] = 1.0
        E[p][n, r] = 1.0
        cols[s] = n * NZ + int(idx[n, k])
    return Wa, ob, E, cols, nvalid, npairs, pw


def _build_program(bloc, nvalid, npairs, pw):
    from concourse import bacc, mybir
    import concourse.tile as tile

    f32 = mybir.dt.float32
    f16 = mybir.dt.float16
    AF = mybir.ActivationFunctionType
    OP = mybir.AluOpType
    nc = bacc.Bacc("TRN2", target_bir_lowering=False, debug=False)

    lw = bloc // NLOAD
    xb_d = [
        nc.declare_dram_parameter(f"xb{i}", [DA, lw], f16, isOutput=False)
        for i in range(NLOAD)
    ]
    Wa_d = nc.declare_dram_parameter("Wa", [DA, npairs * P], f16, isOutput=False)
    ob_d = [
        nc.declare_dram_parameter(f"ob{p}", [pw[p], NZ], f16, isOutput=False)
        for p in range(npairs)
    ]
    E_d = [
        nc.declare_dram_parameter(f"E{p}", [NZ, pw[p]], f16, isOutput=False)
        for p in range(npairs)
    ]
    out_d = nc.declare_dram_parameter("out", [nvalid, bloc], f16, isOutput=True)

    n_blk = bloc // BF

    with tile.TileContext(nc) as tc:
        with (
            tc.tile_pool(name="const", bufs=1) as cpool,
            tc.tile_pool(name="work", bufs=2) as wpool,
            tc.tile_pool(name="stage", bufs=2) as spool,
            tc.tile_pool(name="ps_log", bufs=2, space="PSUM") as ps_log,
            tc.tile_pool(name="ps_den", bufs=1, space="PSUM") as ps_den,
            tc.tile_pool(name="ps_rf", bufs=2, space="PSUM") as ps_rf,
        ):
            Wa_sb = cpool.tile([DA, npairs * P], f16, tag="Wa")
            nc.sync.dma_start(out=Wa_sb[:], in_=Wa_d[:])
            ob_sb, E_sb = [], []
            for p in range(npairs):
                t = cpool.tile([pw[p], NZ], f16, tag=f"ob{p}")
                nc.sync.dma_start(out=t[:], in_=ob_d[p][:])
                ob_sb.append(t)
                t = cpool.tile([NZ, pw[p]], f16, tag=f"E{p}")
                nc.sync.dma_start(out=t[:], in_=E_d[p][:])
                E_sb.append(t)
            xTa_sb = cpool.tile([DA, bloc], f16, tag="xTa")
            load_engs = [nc.sync, nc.gpsimd, nc.scalar]
            for i in range(NLOAD):
                load_engs[i % len(load_engs)].dma_start(
                    out=xTa_sb[:, i * lw:(i + 1) * lw], in_=xb_d[i][:]
                )

            # Superblocks of 2 BF-column blocks, software-pipelined 3 deep:
            # at iteration t the tensor engine runs logits(t), denom(t-1),
            # expand(t-2) — every matmul's inputs were produced a full
            # superblock earlier, so the tensor stream never waits and the
            # PE activity monitor keeps the array un-throttled. Within each
            # phase the pair loop is outer (stationary operand reuse).
            n_sb = n_blk // 2
            exh, den, rh = {}, {}, {}
            stage = {}

            def emit_lg(sb):
                for p in range(npairs):
                    for j in (2 * sb, 2 * sb + 1):
                        lg = ps_log.tile([P, BF], f32, tag="lg")
                        nc.tensor.matmul(
                            lg[:pw[p], :],
                            Wa_sb[:, P * p:P * p + pw[p]],
                            xTa_sb[:, j * BF:(j + 1) * BF],
                            start=True,
                            stop=True,
                        )
                        eh = wpool.tile(
                            [P, BF], f16, bufs=3,
                            tag=f"exh{p}_{j % 2}", name=f"exh{p}",
                        )
                        nc.scalar.activation(eh[:pw[p], :], lg[:pw[p], :], AF.Exp)
                        exh[p, j] = eh

            def emit_den(sb):
                js = (2 * sb, 2 * sb + 1)
                for j in js:
                    den[j] = ps_den.tile([NZ, BF], f32, tag=f"den{j % 2}", name="den")
                for p in range(npairs):
                    for j in js:
                        nc.tensor.matmul(
                            den[j][:, :], ob_sb[p][:], exh[p, j][:pw[p], :],
                            start=(p == 0), stop=(p == npairs - 1),
                        )
                for j in js:
                    rc = wpool.tile([NZ, BF], f32, tag=f"rc{j % 2}", name="rc")
                    nc.vector.reciprocal_approx_fast(out=rc[:], in_=den[j][:])
                    r = wpool.tile([NZ, BF], f16, tag=f"rh{j % 2}", name="rh")
                    nc.vector.tensor_copy(r[:], rc[:])
                    rh[j] = r

            def emit_rf(sb):
                js = (2 * sb, 2 * sb + 1)
                for p in range(npairs):
                    stage[p] = spool.tile(
                        [P, 2 * BF], f16, tag=f"st{p}", name=f"st{p}"
                    )
                    for j in js:
                        rf = ps_rf.tile([P, BF], f32, tag="rf")
                        nc.tensor.matmul(
                            rf[:pw[p], :], E_sb[p][:], rh[j][:],
                            start=True, stop=True,
                        )
                        nc.vector.tensor_tensor(
                            out=stage[p][:pw[p], (j % 2) * BF:(j % 2 + 1) * BF],
                            in0=exh[p, j][:pw[p], :],
                            in1=rf[:pw[p], :],
                            op=OP.mult,
                        )
                # split each pair's store into half-row chunks issued from
                # different engines: each issuing engine owns its own DMA
                # ring, so descriptors spread across the 16 DMA engines.
                store_engs = [nc.sync, nc.gpsimd]
                ei = 0
                bs = js[0] * BF
                for p in range(npairs):
                    for h in range(0, pw[p], 64):
                        he = min(h + 64, pw[p])
                        store_engs[ei % len(store_engs)].dma_start(
                            out=out_d[P * p + h:P * p + he, bs:bs + 2 * BF],
                            in_=stage[p][h:he, :],
                        )
                        ei += 1

            for t in range(n_sb + 2):
                if t < n_sb:
                    emit_lg(t)
                if 1 <= t < n_sb + 1:
                    emit_den(t - 1)
                if t >= 2:
                    emit_rf(t - 2)
    nc.compile()
    return nc


def _install_ntff_hook():
    """Shim antenv.axon_hooks (absent in this image) so trace=True can drive
    NRT profiling through libaxon_pjrt.so. Only used for self-profiling."""
    import types

    try:
        import antenv

        try:
            from antenv.axon_hooks import get_axon_ntff_profile_hook  # noqa: F401

            return True
        except ImportError:
            pass
        if "/root/.axon_site" not in sys.path:
            sys.path.insert(0, "/root/.axon_site")
        from trn_agent_boot.trn_boot import _ntff_profile_via_ctypes

        hook = _ntff_profile_via_ctypes("/opt/axon/libaxon_pjrt.so")
        mod = types.ModuleType("antenv.axon_hooks")
        state = {"hook": hook}
        mod.get_axon_ntff_profile_hook = lambda: state["hook"]
        mod.set_axon_ntff_profile_hook = lambda h: state.update(hook=h)
        sys.modules["antenv.axon_hooks"] = mod
        antenv.axon_hooks = mod
        return hook is not None
    except Exception as e:  # profiling is best-effort; never break the run
        print("ntff hook install failed:", e)
        return False


def kernel(obs, W, b, idx, mask):
    from concourse.bass_utils import run_bass_kernel_spmd

    global LAST_RESULTS
    trace = bool(int(os.environ.get("KBT_TRACE", "0")))
    if trace:
        trace = _install_ntff_hook()
    obs = np.asarray(obs, np.float32)
    Wa, ob, E, cols, nvalid, npairs, pw = _build_consts(W, b, idx, mask)

    nc = _build_program(BLOC, nvalid, npairs, pw)

    consts = {"Wa": Wa}
    for p in range(npairs):
        consts[f"ob{p}"] = ob[p]
        consts[f"E{p}"] = E[p]

    lw = BLOC // NLOAD
    in_maps = []
    for i in range(NCORES):
        xT = np.ascontiguousarray(obs[i * BLOC:(i + 1) * BLOC, :D].T)
        xTa = np.empty((DA, BLOC), np.float16)
        xTa[:D] = xT.astype(np.float16)
        xTa[D] = np.float16(1.0)
        m = dict(consts)
        for j in range(NLOAD):
            m[f"xb{j}"] = np.ascontiguousarray(xTa[:, j * lw:(j + 1) * lw])
        in_maps.append(m)

    br = run_bass_kernel_spmd(nc, in_maps, list(range(NCORES)), trace=trace)
    LAST_RESULTS = br

    # host scatter: compact probs rows -> the (batch-independent) nonzero
    # columns of the zeroed [B, 81*81] output; duplicate (n, idx) pairs sum.
    packed = np.concatenate(
        [np.asarray(br.results[i]["out"])[:nvalid] for i in range(NCORES)], axis=1
    )  # [nvalid, BATCH] fp16
    ucols, first_i, inv = np.unique(cols, return_index=True, return_inverse=True)
    acc = packed[first_i].astype(np.float32)  # [nuniq, BATCH]
    dup = np.setdiff1d(np.arange(nvalid), first_i, assume_unique=False)
    for s in dup:
        acc[inv[s]] += packed[s].astype(np.float32)
    out = np.zeros((BATCH, NZ * NZ), np.float32)
    out[:, ucols] = acc.T
    return out.reshape(BATCH, NZ, NZ)
